# revision 1
# baseline (speedup 1.0000x reference)
"""DenseSNN Trainium2 kernel: 4-layer LIF SNN, T=100 steps, B=128, D=H=2048, C=100.

Strategy
--------
The reference scans timesteps with all 4 layers inside the scan body, but the
dependency structure is feed-forward across layers: layer-l spikes at step t
depend only on layer-(l-1) spikes at steps <= t. So the computation unrolls into
per-layer phases:

    CUR1 = x @ W1 + b1          (batched over all T*B rows)
    S1   = LIF-scan_T(CUR1)     (elementwise in (B,H), sequential in T)
    CUR2 = S1 @ W2 + b2 ; S2 = LIF-scan(CUR2)
    CUR3 = S2 @ W3 + b3 ; S3 = LIF-scan(CUR3)
    CURo = S3 @ Wo + bo ; out = sum_t LIF-scan(CURo)

This turns the tiny per-step GEMMs into full-size GEMMs and makes pure
data-parallelism over batch (16 samples/core on 8 cores) communication-free.

On-chip layout is "transposed activations": [feature -> 16 chunks x 128
partitions, (t,b) -> free axis]. Weight-stationary matmuls (lhsT = W tile in
natural [D,H] layout) keep every tensor in this layout end to end; the host
pre-transposes x and re-assembles the output, so the device never transposes.

Matmuls run in fp8e4 DoubleRow mode (2 contraction k-tiles per instruction,
2x bf16 throughput). Weights are pre-scaled by 512 on the host so their
~1/sqrt(H) magnitudes sit in fp8e4's normal range; the PSUM->SBUF activation
descales by 1/512 and adds the (fp32) bias. Spikes are 0/1 - exact in fp8.
LIF membrane state is fp32 on the vector engine. reset(t) == spike(t-1),
which saves one compare per step.
"""

import numpy as np
import ml_dtypes

import concourse.bass as bass
import concourse.mybir as mybir
import concourse.tile as tile
from concourse import bacc
from concourse.bass_utils import run_bass_kernel_spmd

# Problem constants (hardcoded per contract)
T, B, D, H, C = 100, 128, 2048, 2048, 100
NCORES = 8
BC = B // NCORES          # 16 samples per core
R = T * BC                # 1600 rows (t,b) per core
KC = D // 128             # 16 contraction chunks
KP = KC // 2              # 8 DoubleRow chunk-pairs
HC = H // 128             # 16 output-feature chunks
BETA = 0.9
WSCALE = 512.0            # host-side fp8 weight scale; descaled in activation
NR = 512                  # row-slice width (multiple of BC)
SLICES = [(r0, min(NR, R - r0)) for r0 in range(0, R, NR)]

import os
_DEBUG_SPIKES = bool(os.environ.get("SNN_DEBUG_SPIKES"))
F32 = mybir.dt.float32
BF16 = mybir.dt.bfloat16
F8 = mybir.dt.float8e4
ALU = mybir.AluOpType
ACTF = mybir.ActivationFunctionType
DROW = mybir.MatmulPerfMode.DoubleRow


def _build_nc(n_repeat=1):
    """n_repeat>1 builds a timing variant that runs the whole body N times
    back-to-back (membrane state carries across reps; output is then
    meaningless but instruction mix/timing per rep is identical)."""
    nc = bacc.Bacc("TRN2", target_bir_lowering=False)

    xT_d = nc.dram_tensor("xT", [KC, 128, R], F8, kind="ExternalInput")
    w_d = [
        nc.dram_tensor("w1", [D, H], F8, kind="ExternalInput"),
        nc.dram_tensor("w2", [H, H], F8, kind="ExternalInput"),
        nc.dram_tensor("w3", [H, H], F8, kind="ExternalInput"),
    ]
    wo_d = nc.dram_tensor("wo", [H, 128], F8, kind="ExternalInput")  # C pad to 128
    bias_d = nc.dram_tensor("biases", [128, 3 * HC], F32, kind="ExternalInput")
    bo_d = nc.dram_tensor("biaso", [C, 1], F32, kind="ExternalInput")
    out_d = nc.dram_tensor("out", [C, BC], F32, kind="ExternalOutput")

    with tile.TileContext(nc) as tc:
        with (
            tc.tile_pool(name="spool", bufs=2) as spool,
            tc.tile_pool(name="wpool", bufs=2) as wpool,
            tc.tile_pool(name="xpool", bufs=2) as xpool,
            tc.tile_pool(name="cpool", bufs=2) as cpool,
            tc.tile_pool(name="small", bufs=1) as small,
            tc.tile_pool(name="pspool", bufs=8, space="PSUM") as pspool,
        ):
            # Persistent big tensors (fp8: spikes exactly representable)
            S1 = spool.tile([128, KC * R], F8, tag="S")
            S2 = spool.tile([128, KC * R], F8, tag="S")
            S3 = spool.tile([128, KC * R], F8, tag="S")  # reuses S1's slot
            w_sb = [
                wpool.tile([128, KC * H], F8, tag="W", name=f"w{i}_sb")
                for i in range(3)
            ]                                            # w3 reuses w1's slot
            wo_sb = small.tile([128, KC * 128], F8)

            # LIF state. Membranes are bf16 (2x DVE mode) and ping-pong
            # buffered: the spike compare (tensor_scalar, 2x) writes a
            # contiguous bf16 ring and gpsimd tensor_copy scatters it into
            # the fp8 S tile, so every DVE operand is contiguous 16-bit.
            stm = small.tile([128, 9 * 256], BF16)   # 3x(2 pp + 1 tmp)
            mem_pp = [
                [stm[:, (2 * li + pp) * 256:(2 * li + pp + 1) * 256]
                 for pp in range(2)]
                for li in range(3)
            ]
            mem_t = [
                stm[:, (6 + li) * 256:(7 + li) * 256] for li in range(3)
            ]
            sring = small.tile([128, 3 * 4 * 256], BF16)  # 4-deep/layer
            zbf = small.tile([128, 256], BF16)            # zeros (t=0 s_prev)

            # fp32 small state: biases + output layer
            st = small.tile([128, 144], F32)
            bias_sb = st[:, 0:48]               # [128, 48] = 3 layers x 16 chunks
            memo = st[:100, 48:64]              # [100, 16]
            ssum = st[:100, 64:80]
            zo = st[:100, 80:96]                # zeros (Lo t=0 s_prev)
            so_ring = st[:100, 96:128]          # [100, 32] ping-pong spikes
            bo_sb = st[:100, 128:129]           # [100, 1]

            nc.gpsimd.memset(st[:], 0.0)
            nc.gpsimd.memset(stm[:], 0.0)
            nc.gpsimd.memset(sring[:], 0.0)
            nc.gpsimd.memset(zbf[:], 0.0)
            nc.sync.dma_start(bias_sb, bias_d[:])
            nc.sync.dma_start(bo_sb, bo_d[:])
            for kc in range(KC):
                nc.sync.dma_start(
                    wo_sb[:, kc * 128:(kc + 1) * 128],
                    wo_d[kc * 128:(kc + 1) * 128, :],
                )

            def body():
                def dense_layer(li, rhs_of, S_out):
                    """One hidden layer: matmul all row-slices + LIF scan over T."""
                    w = w_sb[li]
                    for kc in range(KC):
                        nc.sync.dma_start(
                            w[:, kc * H:(kc + 1) * H],
                            w_d[li][kc * 128:(kc + 1) * 128, :],
                        )
                    w3d = w.rearrange("p (c h) -> p c h", c=KC)
                    S_out3 = S_out.rearrange("p (c r) -> p c r", c=KC)
                    mpp = mem_pp[li]
                    mt = mem_t[li]
                    ring = sring[:, li * 1024:(li + 1) * 1024]
                    for r0, nr in SLICES:
                        rhs = rhs_of(r0, nr)
                        nst = nr // BC
                        # cur is written t-major: [p, (step, feat-chunk, b)] so
                        # each LIF step reads a contiguous [128, 256] slice
                        cur = cpool.tile(
                            [128, (NR // BC) * 256], BF16, tag="cur", name="cur"
                        )
                        curT = cur.rearrange("p (t cb) -> p t cb", cb=256)
                        for hc in range(HC):
                            ps = pspool.tile([128, NR], F32, tag="ps", name="ps")
                            for kp in range(KP):
                                nc.tensor.matmul(
                                    ps[:, :nr],
                                    w3d[:, 2 * kp:2 * kp + 2,
                                        hc * 128:hc * 128 + 128],
                                    rhs(kp),
                                    start=(kp == 0),
                                    stop=(kp == KP - 1),
                                    perf_mode=DROW,
                                )
                            nc.scalar.activation(
                                curT[:, :nst, hc * BC:(hc + 1) * BC],
                                ps[:, :nr].rearrange("p (t b) -> p t b", b=BC),
                                ACTF.Identity,
                                bias=bias_sb[:, li * HC + hc: li * HC + hc + 1],
                                scale=1.0 / WSCALE,
                            )
                        for tl in range(nst):
                            t = r0 // BC + tl
                            cur_t = cur[:, tl * 256:(tl + 1) * 256]
                            sp_c = (
                                zbf[:] if t == 0
                                else ring[:, ((t - 1) % 4) * 256:((t - 1) % 4 + 1) * 256]
                            )
                            sn_c = ring[:, (t % 4) * 256:(t % 4 + 1) * 256]
                            m_prev = mpp[(t - 1) % 2]
                            m_cur = mpp[t % 2]
                            # tmp = beta*mem + cur      (bf16 contiguous, 2x)
                            nc.vector.scalar_tensor_tensor(
                                mt, m_prev, BETA, cur_t, ALU.mult, ALU.add
                            )
                            # mem_new = tmp - s_prev  (reset-by-subtraction)
                            nc.vector.tensor_tensor(m_cur, mt, sp_c, ALU.subtract)
                            # spike = mem_new > 1  (single-tensor op, 2x/4x)
                            nc.vector.tensor_scalar(
                                sn_c, m_cur, 1.0, None, ALU.is_gt
                            )
                            # scatter bf16 ring -> fp8 S (c-major) on gpsimd,
                            # off the DVE scan chain
                            nc.gpsimd.tensor_copy(
                                out=S_out3[:, :, t * BC:(t + 1) * BC],
                                in_=sn_c.rearrange("p (c b) -> p c b", c=KC),
                            )

                # ---- Layer 1: rhs streamed from HBM (x^T, host-pretransposed)
                def rhs_layer1(r0, nr):
                    xin = xpool.tile([128, KC * NR], F8, tag="xin", name="xin")
                    for kc in range(KC):
                        nc.sync.dma_start(
                            xin[:, kc * nr:(kc + 1) * nr], xT_d[kc][:, r0:r0 + nr]
                        )
                    xin3 = xin[:, : KC * nr].rearrange("p (c r) -> p c r", c=KC)
                    return lambda kp: xin3[:, 2 * kp:2 * kp + 2, :]

                dense_layer(0, rhs_layer1, S1)

                # ---- Layers 2, 3: rhs from previous layer's spikes in SBUF
                def rhs_from(S_in):
                    S_in3 = S_in.rearrange("p (c r) -> p c r", c=KC)
                    def f(r0, nr):
                        return lambda kp: S_in3[:, 2 * kp:2 * kp + 2, r0:r0 + nr]
                    return f

                dense_layer(1, rhs_from(S1), S2)
                dense_layer(2, rhs_from(S2), S3)

                # ---- Output layer + spike-count accumulation
                S3_3 = S3.rearrange("p (c r) -> p c r", c=KC)
                wo3d = wo_sb.rearrange("p (c h) -> p c h", c=KC)
                for r0, nr in SLICES:
                    ps = pspool.tile([128, NR], F32, tag="ps", name="pso")
                    for kp in range(KP):
                        nc.tensor.matmul(
                            ps[:, :nr],
                            wo3d[:, 2 * kp:2 * kp + 2, :],
                            S3_3[:, 2 * kp:2 * kp + 2, r0:r0 + nr],
                            start=(kp == 0),
                            stop=(kp == KP - 1),
                            perf_mode=DROW,
                        )
                    curo = cpool.tile([128, NR], F32, tag="curo", name="curo")
                    curo_f = curo[:100, :nr]
                    nc.scalar.activation(
                        curo_f, ps[:100, :nr], ACTF.Identity,
                        bias=bo_sb, scale=1.0 / WSCALE,
                    )
                    for tl in range(nr // BC):
                        t = r0 // BC + tl
                        cur_t = curo_f[:, tl * BC:(tl + 1) * BC]
                        so_prev = zo if t == 0 else so_ring[:, (1 - t % 2) * BC:(2 - t % 2) * BC]
                        so_new = so_ring[:, (t % 2) * BC:(t % 2 + 1) * BC]
                        nc.vector.scalar_tensor_tensor(
                            memo, memo, BETA, cur_t, ALU.mult, ALU.add
                        )
                        nc.vector.scalar_tensor_tensor(
                            so_new, memo, 1.0, so_prev, ALU.subtract, ALU.is_gt
                        )
                        nc.vector.tensor_tensor(memo, memo, so_prev, ALU.subtract)
                        nc.vector.tensor_tensor(ssum, ssum, so_new, ALU.add)


            for _rep in range(n_repeat):
                body()

            nc.sync.dma_start(out_d[:], ssum)

            if _DEBUG_SPIKES:
                for nm, S in (("s1_dbg", S1), ("s2_dbg", S2), ("s3_dbg", S3)):
                    sd = nc.dram_tensor(nm, [128, KC * R], F8, kind="ExternalOutput")
                    nc.sync.dma_start(sd[:], S[:])

    nc.compile()
    return nc


_NC_CACHE = None


def _get_nc():
    global _NC_CACHE
    if _NC_CACHE is None:
        _NC_CACHE = _build_nc()
    return _NC_CACHE


def make_in_maps(x_seq, W1, b1, W2, b2, W3, b3, Wo, bo):
    f8 = ml_dtypes.float8_e4m3
    w1 = np.ascontiguousarray((W1 * WSCALE).astype(f8))
    w2 = np.ascontiguousarray((W2 * WSCALE).astype(f8))
    w3 = np.ascontiguousarray((W3 * WSCALE).astype(f8))
    wo_pad = np.zeros((H, 128), np.float32)
    wo_pad[:, :C] = Wo * WSCALE
    wo = np.ascontiguousarray(wo_pad.astype(f8))
    biases = np.concatenate(
        [b.reshape(HC, 128).T for b in (b1, b2, b3)], axis=1
    ).astype(np.float32)                       # [128, 48]
    biases = np.ascontiguousarray(biases)
    bo_a = np.ascontiguousarray(bo.reshape(C, 1).astype(np.float32))
    in_maps = []
    for c in range(NCORES):
        xs = x_seq[:, c * BC:(c + 1) * BC, :]              # [T, BC, D]
        xT = xs.transpose(2, 0, 1).reshape(KC, 128, R)     # [D,(t,b)] chunked
        in_maps.append({
            "xT": np.ascontiguousarray(xT.astype(f8)),
            "w1": w1, "w2": w2, "w3": w3, "wo": wo,
            "biases": biases, "biaso": bo_a,
        })
    return in_maps


def kernel(x_seq, W1, b1, W2, b2, W3, b3, Wo, bo):
    nc = _get_nc()
    in_maps = make_in_maps(x_seq, W1, b1, W2, b2, W3, b3, Wo, bo)
    res = run_bass_kernel_spmd(nc, in_maps, core_ids=list(range(NCORES)))
    outs = [res.results[c]["out"] for c in range(NCORES)]   # each [C, BC]
    return np.concatenate([o.T for o in outs], axis=0).astype(np.float32)

